# revision 1
# baseline (speedup 1.0000x reference)
"""ALIGN module kernel for 8 TRN2 NeuronCores (vocab-parallel).

Reference computation (B=4, S=576, Dv=1024, Dl=4096, V=32000):
    x  = vision_feats @ W1_w.T + W1_b          # [T=2304, Dl]
    xn = layernorm(x)                          # over Dl, no affine
    P  = softmax(xn @ W2_w.T, axis=-1)         # [T, V]
    F  = P @ llm_token_embed                   # [T, Dl]

Sharding: vocab dim of W2_w / llm_token_embed split across the 8 cores
(4000 rows each, zero-padded to 4096). Stage A (W1 + LN) is token-parallel
with STRIDED ownership: core c owns tokens {sb0 + tch*c + i} of each
superblock, so every superblock is AllGathered separately (4 small AGs,
sized [256, 768, 768, 512]) and each AG overlaps compute: AG0 is tiny and
early, later AGs hide under phase-B matmuls.

W1 and its bias are COLUMN-CENTERED on the host (W1 -= mean over the Dl
output dim), which makes the LayerNorm mean identically zero: stage A only
needs E[x^2] (one ones-vector matmul reduction per chunk), an Rsqrt, a
rank-1 broadcast matmul, and a single elementwise multiply. Stage A
computes xT directly (lhsT=W1T-tile, rhs=visionT) so no PE transposes are
needed anywhere.

Softmax needs no max-subtraction (logits are ~N(0,1), |logit| < ~6): each
core computes exp(logits_loc); the zero pad rows contribute exactly
exp(0)=1 each, masked out of the denominator by the ones_v mask. The
denominator rides e-group 0's ReduceScatter as an extra column; each core
only divides the token slices it owns after the F ReduceScatter. The last
e-group of the last superblock is host-reduced to kill the RS tail.

Pool layout: phase-B-critical pools (w2, xnt, pt, lp-PSUM) are allocated
BEFORE the stage-A pools so phase B's first loads/matmuls are not gated on
stage-A pool release; eb/fs/fo and the remaining PSUM pools allocate into
the space stage A frees.
"""

import os
import sys

for _p in ("/opt/trn_rl_repo", "/root/.axon_site/_ro/trn_rl_repo"):
    if os.path.isdir(_p) and _p not in sys.path:
        sys.path.insert(0, _p)

import numpy as np
import ml_dtypes

from concourse import bass, bacc, mybir, tile
from concourse.bass_utils import run_bass_kernel_spmd

BF16NP = ml_dtypes.bfloat16
F32 = mybir.dt.float32
BF16 = mybir.dt.bfloat16

N_CORES = 8
T = 2304          # total tokens (B*S)
DV = 1024
DL = 4096
V_PAD = 4096      # padded vocab rows per core (4000 real + 96 zero pads)
NVT = V_PAD // 128  # 32 vocab tiles per core
NJ = DL // 128      # 32 contraction tiles
NK = DV // 128      # 8 stage-A contraction tiles

# token superblocks; each is AllGathered separately and owned strided
SBS = [(0, 256), (256, 768), (1024, 768), (1792, 512)]
TSB_MAX = 768
TCHS = [sbn // N_CORES for _, sbn in SBS]      # per-core chunk sizes
TCH_MAX = max(TCHS)
TCH_OFF = [sum(TCHS[:i]) for i in range(len(SBS))]
T_LOC = sum(TCHS)          # 288 tokens per core total
EC = 512          # matmul2 embedding-chunk width (SBUF tile)
N_EC = DL // EC   # 8 e-chunks
EG = 2            # e-chunks per ReduceScatter group
EGW = EC * EG     # 1024 columns per RS

_NC_CACHE = None


def c1s(sbn):
    # matmul1 moving-dim chunking of a superblock: two half-superblock
    # chunks, each spanning 4 core blocks of the block-major xnt
    return [(0, sbn // 2), (sbn // 2, sbn // 2)]


def build():
    nc = bacc.Bacc("TRN2", target_bir_lowering=False, debug=False,
                   num_devices=N_CORES)
    rg = [list(range(N_CORES))]

    visionT = nc.dram_tensor("visionT", [DV, T_LOC], BF16, kind="ExternalInput")
    w1t = nc.dram_tensor("w1t", [DV, DL], BF16, kind="ExternalInput")
    w1b = nc.dram_tensor("w1b", [128, NJ], F32, kind="ExternalInput")
    # [vt][p][j][vi]: per-partition unit-stride 8KB runs
    w2t = nc.dram_tensor("w2t", [NVT, 128, NJ, 128], BF16, kind="ExternalInput")
    # [e][p][vt][n]: per-partition unit-stride 16KB runs
    emb = nc.dram_tensor("emb", [N_EC, 128, NVT, EC], BF16,
                         kind="ExternalInput")
    ones_v = nc.dram_tensor("ones_v", [128, NVT, 1], BF16, kind="ExternalInput")
    out = nc.dram_tensor("out", [T // N_CORES, DL], F32, kind="ExternalOutput")
    # last e-group of the last superblock skips its ReduceScatter: each core
    # ships its partial numerator (and denominator) and the host reduces.
    out2 = nc.dram_tensor("out2", [SBS[-1][1], EGW], BF16, kind="ExternalOutput")
    out3 = nc.dram_tensor("out3", [SBS[-1][1], 1], BF16, kind="ExternalOutput")

    from contextlib import ExitStack
    with tile.TileContext(nc) as tc, ExitStack() as ctx:
        consts = ctx.enter_context(tc.tile_pool(name="consts", bufs=1))
        dram = ctx.enter_context(tc.tile_pool(name="dram", bufs=1, space="DRAM"))
        dram_rs = ctx.enter_context(tc.tile_pool(name="dram_rs", bufs=4, space="DRAM"))
        # phase-B-critical pools allocated BEFORE stage A so phase B's first
        # loads/matmuls are not gated on stage-A pool release
        w2_p = ctx.enter_context(tc.tile_pool(name="w2_p", bufs=3))
        xnt_p = ctx.enter_context(tc.tile_pool(name="xnt_p", bufs=1))
        pt_p = ctx.enter_context(tc.tile_pool(name="pt_p", bufs=1))
        l_ps = ctx.enter_context(tc.tile_pool(name="l_ps", bufs=4, space="PSUM"))
        if True:
            onesv_sb = consts.tile([128, NVT, 1], BF16)
            nc.sync.dma_start(onesv_sb, ones_v[:])

            # per-superblock AllGather buffers; byte layout of each block is
            # xnT row-major [DL, tch] per core
            ag_ins = []
            ag_outs = []
            for si in range(len(SBS)):
                tch = TCHS[si]
                gi = dram.tile([DL * tch], BF16, tag=f"ag_in_{si}",
                               name=f"ag_in_{si}")
                go = dram.tile([N_CORES * DL * tch], BF16,
                               addr_space="Shared", tag=f"ag_out_{si}",
                               name=f"ag_out_{si}")
                ag_ins.append(gi)
                ag_outs.append(go)

            # ---------------- Stage A: xT = W1T.T-tiles @ visionT, LN cols
            with ExitStack() as actx:
                sa = actx.enter_context(tc.tile_pool(name="stageA", bufs=1))
                sa2 = actx.enter_context(tc.tile_pool(name="stageA2", bufs=2))
                psa = actx.enter_context(tc.tile_pool(name="psumA", bufs=1, space="PSUM"))

                vt_sb = sa.tile([128, NK, T_LOC], BF16)
                for k in range(NK):
                    nc.sync.dma_start(
                        vt_sb[:, k, :], visionT[128 * k:128 * (k + 1), :])
                w1t_sb = sa.tile([128, NK, DL], BF16)
                for mb in range(8):
                    for k in range(NK):
                        nc.sync.dma_start(
                            w1t_sb[:, k, 512 * mb:512 * (mb + 1)],
                            w1t[128 * k:128 * (k + 1),
                                512 * mb:512 * (mb + 1)])
                b_cols = sa.tile([128, NJ], F32)
                nc.sync.dma_start(b_cols, w1b[:])
                onescol = sa.tile([128, 1], BF16)
                nc.vector.memset(onescol, 1.0)
                onesrow_f = sa.tile([1, 128], F32)
                nc.vector.memset(onesrow_f, 1.0)
                eps_sc = sa.tile([1, 1], F32)
                nc.vector.memset(eps_sc, 1e-5)

                def bc(t, tch):
                    # [128, tch] -> [128, NJ, tch] stride-0 broadcast
                    return bass.AP(
                        tensor=t.tensor, offset=t.offset,
                        ap=[list(t.ap[0]), [0, NJ], [1, tch]])

                sa_state = []

                def sa_finish(si):
                    # broadcast rstd, apply, ship, AllGather — issued one
                    # chunk late so the PE never waits on the DVE chain
                    xt, rstd_row, tch = sa_state[si]
                    rstdb_p = psa.tile([128, TCH_MAX], F32, tag="rstdb",
                                       name=f"rstdb_{si}")
                    nc.tensor.matmul(rstdb_p[:, :tch], lhsT=onesrow_f,
                                     rhs=rstd_row[:, :tch])
                    rstdb = sa2.tile([128, TCH_MAX], BF16, tag="rstdb_sb",
                                     name=f"rstdb_sb_{si}")
                    nc.vector.tensor_copy(out=rstdb[:, :tch],
                                          in_=rstdb_p[:, :tch])
                    xn_ch = sa2.tile([128, NJ, TCH_MAX], BF16, tag="sq",
                                     name=f"xn_{si}")
                    nc.vector.tensor_mul(out=xn_ch[:, :, :tch],
                                         in0=xt[:, :, :tch],
                                         in1=bc(rstdb, tch))
                    for q in range(4):
                        nc.sync.dma_start(
                            bass.AP(tensor=ag_ins[si].tensor,
                                    offset=(ag_ins[si].offset
                                            + q * 8 * tch),
                                    ap=[[NJ * tch, 128], [1, 8 * tch]]),
                            xn_ch[:, 8 * q:8 * (q + 1), :tch])
                    nc.gpsimd.collective_compute(
                        "AllGather", mybir.AluOpType.bypass,
                        replica_groups=rg,
                        ins=[ag_ins[si].opt()], outs=[ag_outs[si].opt()])

                for si in range(len(SBS)):
                    tch = TCHS[si]
                    c0 = TCH_OFF[si]
                    # xT for this chunk: [dl-part, m, token]
                    xt = sa2.tile([128, NJ, TCH_MAX], BF16, tag="xt",
                                  name=f"xt_{si}")
                    sq = sa2.tile([128, NJ, TCH_MAX], BF16, tag="sq",
                                  name=f"sq_{si}")
                    for m in range(NJ):
                        xp = psa.tile([128, TCH_MAX], F32, tag="xp",
                                      name=f"xp_{si}_{m}", bufs=2)
                        for k in range(NK):
                            nc.tensor.matmul(
                                xp[:, :tch],
                                lhsT=w1t_sb[:, k, 128 * m:128 * (m + 1)],
                                rhs=vt_sb[:, k, c0:c0 + tch],
                                start=(k == 0), stop=(k == NK - 1))
                        nc.scalar.activation(
                            out=xt[:, m, :tch], in_=xp[:, :tch],
                            func=mybir.ActivationFunctionType.Identity,
                            bias=b_cols[:, m:m + 1])
                        nc.vector.tensor_mul(out=sq[:, m, :tch],
                                             in0=xt[:, m, :tch],
                                             in1=xt[:, m, :tch])
                        # previous chunk's broadcast+apply+AG (the PE-DVE
                        # ping-pong hides under this chunk's matmuls)
                        if si > 0 and m == 16:
                            sa_finish(si - 1)
                    # E[x^2] per token via ones-vector matmul reduction
                    # (mean is exactly 0: W1/b are host-centered)
                    s2p = psa.tile([1, TCH_MAX], F32, tag="s2",
                                   name=f"s2_{si}")
                    for m in range(NJ):
                        nc.tensor.matmul(s2p[:, :tch], lhsT=onescol,
                                         rhs=sq[:, m, :tch],
                                         start=(m == 0), stop=(m == NJ - 1))
                    msq_row = sa2.tile([1, TCH_MAX], F32, tag="msq",
                                       name=f"msq_{si}", bufs=1)
                    nc.vector.tensor_scalar(
                        out=msq_row[:, :tch], in0=s2p[:, :tch],
                        scalar1=1.0 / DL, scalar2=None,
                        op0=mybir.AluOpType.mult)
                    sd_row = sa2.tile([1, TCH_MAX], F32, tag="sd",
                                      name=f"sd_{si}", bufs=1)
                    nc.scalar.activation(
                        out=sd_row[:, :tch], in_=msq_row[:, :tch],
                        func=mybir.ActivationFunctionType.Sqrt,
                        bias=eps_sc)
                    rstd_row = sa2.tile([1, TCH_MAX], F32, tag="rstd",
                                        name=f"rstd_{si}")
                    nc.vector.reciprocal(out=rstd_row[:, :tch],
                                         in_=sd_row[:, :tch])
                    sa_state.append((xt, rstd_row, tch))
                sa_finish(len(SBS) - 1)

            # ---------------- Phase B
            eb_p = ctx.enter_context(tc.tile_pool(name="eb_p", bufs=2))
            fs_p = ctx.enter_context(tc.tile_pool(name="fs_p", bufs=2))
            fo_p = ctx.enter_context(tc.tile_pool(name="fo_p", bufs=1))
            small = ctx.enter_context(tc.tile_pool(name="small", bufs=2))
            s_ps = ctx.enter_context(tc.tile_pool(name="s_ps", bufs=1, space="PSUM"))
            f_ps = ctx.enter_context(tc.tile_pool(name="f_ps", bufs=3, space="PSUM"))
            if True:

                def make_xnt(si):
                    tch = TCHS[si]
                    # xnt is stored core-block-major [p, c, j, i]: both the
                    # AG payload and the SBUF tile are contiguous per
                    # (partition, block), so each block is one clean DMA
                    # with 6KB-per-partition runs
                    xnt = xnt_p.tile([128, N_CORES, NJ, TCH_MAX], BF16,
                                     tag="xnt", name=f"xnt_{si}")
                    for c in range(N_CORES):
                        off = ag_outs[si].offset + c * 128 * NJ * tch
                        nc.sync.dma_start(
                            xnt[:, c, :, :tch],
                            bass.AP(tensor=ag_outs[si].tensor, offset=off,
                                    ap=[[NJ * tch, 128], [1, NJ * tch]]))
                    return xnt

                def xnt_rhs(xnt, j, c0, cw, tch):
                    # tokens [c0, c0+cw) as c-blocks: [p][c][i] with j fixed
                    nc_blk = cw // tch
                    return bass.AP(
                        tensor=xnt.tensor,
                        offset=(xnt.offset + (c0 // tch) * (NJ * TCH_MAX)
                                + j * TCH_MAX),
                        ap=[list(xnt.ap[0]), [NJ * TCH_MAX, nc_blk],
                            [1, tch]])

                xnt = make_xnt(0)
                for si, (sb0, sbn) in enumerate(SBS):
                    n_tt = sbn // 128
                    # matmul1: logitsT per v-tile, exp -> pt
                    pt = pt_p.tile([128, NVT, TSB_MAX], BF16, tag="pt",
                                   name=f"pt_{si}")
                    NQ = NJ // 2
                    for vt in range(NVT):
                        w2q = []
                        for q in range(2):
                            wq = w2_p.tile([128, NQ, 128], BF16, tag="w2",
                                           name=f"w2_{si}_{vt}_{q}")
                            nc.sync.dma_start(
                                wq, w2t[vt][:, NQ * q:NQ * (q + 1), :])
                            w2q.append(wq)
                        for c0, cw in c1s(sbn):
                            lp = l_ps.tile([128, 512], F32, tag="lp",
                                           name=f"lp_{si}_{vt}_{c0}")
                            for j in range(NJ):
                                nc.tensor.matmul(
                                    lp[:, :cw],
                                    lhsT=w2q[j // NQ][:, j % NQ, :],
                                    rhs=xnt_rhs(xnt, j, c0, cw, TCHS[si]),
                                    start=(j == 0), stop=(j == NJ - 1))
                            nc.scalar.activation(
                                out=pt[:, vt, c0:c0 + cw], in_=lp[:, :cw],
                                func=mybir.ActivationFunctionType.Exp)

                    # queue next superblock's xnt loads ahead of matmul2
                    next_xnt = make_xnt(si + 1) if si + 1 < len(SBS) else None

                    rs_rows = sbn // N_CORES

                    # matmul2: F_partial = pt.T @ emb, RS per e-group, local
                    # divide on owned rows. Last superblock's last group is
                    # host-reduced to shrink the exposed RS tail.
                    egroups = [2, 2, 2, 2]
                    last_host = si == len(SBS) - 1
                    row_off = sb0 // N_CORES
                    col = 0
                    e = 0
                    # denominator s[t] = sum over real v rows of pt (onesv
                    # masks the pads); done as N=1 matmul chains up front so
                    # it never interrupts the F-matmul pipeline. s gets its
                    # own tiny contiguous ReduceScatter (a strided column in
                    # the big RS buffer costs 2-byte DMA descriptors).
                    rs_in_s = dram_rs.tile([TSB_MAX, 1], BF16, tag="rsin_s",
                                           name=f"rsin_s_{si}")
                    for tt in range(n_tt):
                        sp = s_ps.tile([128, 1], F32, tag="sp",
                                       name=f"sp_{si}_{tt}")
                        for vt in range(NVT):
                            nc.tensor.matmul(
                                sp, lhsT=pt[:, vt, 128 * tt:128 * (tt + 1)],
                                rhs=onesv_sb[:, vt, :],
                                start=(vt == 0), stop=(vt == NVT - 1))
                        ss = fs_p.tile([128, 1], BF16, tag="fs",
                                       name=f"ss_{si}_{tt}")
                        nc.scalar.activation(
                            out=ss, in_=sp,
                            func=mybir.ActivationFunctionType.Identity)
                        nc.sync.dma_start(
                            rs_in_s[128 * tt:128 * (tt + 1), :], ss)
                        if last_host:
                            nc.sync.dma_start(
                                out3[128 * tt:128 * (tt + 1), :], ss)
                    rs_out_s = dram_rs.tile([TSB_MAX // N_CORES, 1], BF16,
                                            tag="rsout_s",
                                            name=f"rsout_s_{si}")
                    nc.gpsimd.collective_compute(
                        "ReduceScatter", mybir.AluOpType.add,
                        replica_groups=rg,
                        ins=[rs_in_s[:sbn].opt()],
                        outs=[rs_out_s[:rs_rows].opt()])
                    fo_s = small.tile([TSB_MAX // N_CORES, 1], BF16,
                                      tag="fo_s", name=f"fo_s_{si}")
                    nc.sync.dma_start(fo_s[:rs_rows], rs_out_s[:rs_rows])
                    rsg = small.tile([TSB_MAX // N_CORES, 1], F32,
                                     tag="rsg", name=f"rsg_{si}")
                    nc.vector.reciprocal(out=rsg[:rs_rows],
                                         in_=fo_s[:rs_rows])
                    for gi, gsz in enumerate(egroups):
                        gw = gsz * EC
                        rs_in = dram_rs.tile([TSB_MAX, gw], BF16,
                                             tag="rsin",
                                             name=f"rsin_{si}_{gi}")
                        for ei in range(gsz):
                            eb = eb_p.tile([128, NVT, EC], BF16, tag="eb",
                                           name=f"eb_{si}_{e}")
                            # split the 4MB load into 8 sub-DMAs so it
                            # spreads across queues instead of serializing
                            # on one
                            for sp8 in range(8):
                                nc.sync.dma_start(
                                    eb[:, 4 * sp8:4 * (sp8 + 1), :],
                                    emb[e][:, 4 * sp8:4 * (sp8 + 1), :])
                            for tt in range(n_tt):
                                fp = f_ps.tile([128, EC], F32, tag="fp",
                                               name=f"fp_{si}_{e}_{tt}")
                                for vt in range(NVT):
                                    nc.tensor.matmul(
                                        fp,
                                        lhsT=pt[:, vt, 128 * tt:128 * (tt + 1)],
                                        rhs=eb[:, vt, :],
                                        start=(vt == 0), stop=(vt == NVT - 1))
                                fs = fs_p.tile([128, EC], BF16, tag="fs2",
                                               name=f"fs_{si}_{e}_{tt}")
                                nc.scalar.activation(
                                    out=fs, in_=fp,
                                    func=mybir.ActivationFunctionType.Identity)
                                if last_host and gi == len(egroups) - 1:
                                    nc.sync.dma_start(
                                        out2[128 * tt:128 * (tt + 1),
                                             EC * ei:EC * (ei + 1)], fs)
                                else:
                                    nc.sync.dma_start(
                                        rs_in[128 * tt:128 * (tt + 1),
                                              EC * ei:EC * (ei + 1)], fs)
                            e += 1
                        if last_host and gi == len(egroups) - 1:
                            col += gsz * EC
                            continue
                        rs_out = dram_rs.tile([TSB_MAX // N_CORES, gw],
                                              BF16, tag="rsout",
                                              name=f"rsout_{si}_{gi}")
                        nc.gpsimd.collective_compute(
                            "ReduceScatter", mybir.AluOpType.add,
                            replica_groups=rg,
                            ins=[rs_in[:sbn].opt()],
                            outs=[rs_out[:rs_rows].opt()])
                        fo = fo_p.tile([TSB_MAX // N_CORES, EGW + 1],
                                       BF16, tag="fo", name=f"fo_{si}_{gi}")
                        nc.sync.dma_start(fo[:rs_rows, :gw],
                                          rs_out[:rs_rows])
                        fw = gsz * EC
                        nc.vector.tensor_scalar_mul(
                            out=fo[:rs_rows, :fw], in0=fo[:rs_rows, :fw],
                            scalar1=rsg[:rs_rows])
                        nc.gpsimd.dma_start(
                            out[row_off:row_off + rs_rows, col:col + fw],
                            fo[:rs_rows, :fw])
                        col += fw
                    xnt = next_xnt

    nc.compile()
    return nc


def _get_nc():
    global _NC_CACHE
    if _NC_CACHE is None:
        _NC_CACHE = build()
    return _NC_CACHE


def _prep_in_maps(vision_feats, W1_w, W1_b, W2_w, llm_token_embed):
    vf = np.ascontiguousarray(np.asarray(vision_feats, np.float32)).reshape(
        T, DV)
    W1 = np.asarray(W1_w, np.float32)
    b1 = np.asarray(W1_b, np.float32).reshape(DL)
    # column-center W1/b over the Dl output dim: makes the LN mean exactly 0
    W1 = W1 - W1.mean(axis=0, keepdims=True)
    b1 = np.ascontiguousarray((b1 - b1.mean()).reshape(NJ, 128).T)
    W2 = np.asarray(W2_w, np.float32)
    E = np.asarray(llm_token_embed, np.float32)

    w1t = np.ascontiguousarray(W1.T).astype(BF16NP)
    v_loc = 32000 // N_CORES
    in_maps = []
    for c in range(N_CORES):
        # strided ownership: core c owns tokens {sb0 + tch*c + i}
        tok = np.concatenate([
            np.arange(sb0 + TCHS[si] * c, sb0 + TCHS[si] * (c + 1))
            for si, (sb0, _) in enumerate(SBS)])
        vT = np.ascontiguousarray(vf[tok].T).astype(BF16NP)
        w2p = np.zeros((V_PAD, DL), np.float32)
        w2p[:v_loc] = W2[v_loc * c:v_loc * (c + 1)]
        # [vt, p, j, vi] with p = d % 128, j = d // 128, vi = v % 128
        w2tt = w2p.T.reshape(NJ, 128, NVT, 128).transpose(2, 1, 0, 3).astype(
            BF16NP)
        ep = np.zeros((V_PAD, DL), np.float32)
        ep[:v_loc] = E[v_loc * c:v_loc * (c + 1)]
        # [e, p, vt, n] with p = v % 128, vt = v // 128, n = d % EC
        ebt = ep.reshape(NVT, 128, N_EC, EC).transpose(2, 1, 0, 3).astype(
            BF16NP)
        onesv = np.zeros((128, NVT, 1), np.float32)
        for vt in range(NVT):
            for p in range(128):
                if 128 * vt + p < v_loc:
                    onesv[p, vt, 0] = 1.0
        in_maps.append({
            "visionT": vT,
            "w1t": w1t,
            "w1b": b1,
            "w2t": np.ascontiguousarray(w2tt),
            "emb": np.ascontiguousarray(ebt),
            "ones_v": onesv.astype(BF16NP),
        })
    return in_maps


def run_on_cores(in_maps, trace=False, **kwargs):
    nc = _get_nc()
    return run_bass_kernel_spmd(nc, in_maps, core_ids=list(range(N_CORES)),
                                trace=trace, **kwargs)


def assemble(core_results):
    full = np.empty((T, DL), np.float32)
    for c in range(N_CORES):
        o = np.asarray(core_results[c]["out"])  # [T // N_CORES, DL]
        for sb0, sbn in SBS:
            rs_rows = sbn // N_CORES
            full[sb0 + rs_rows * c:sb0 + rs_rows * (c + 1)] = \
                o[sb0 // N_CORES:sb0 // N_CORES + rs_rows]
    # host-reduced last e-group of the last superblock
    sb0, sbn = SBS[-1]
    num = sum(np.asarray(r["out2"]).astype(np.float32)
              for r in core_results)
    den = sum(np.asarray(r["out3"]).astype(np.float32)
              for r in core_results)
    full[sb0:sb0 + sbn, DL - EGW:] = num[:sbn] / den[:sbn]
    return full.reshape(4, 576, DL)


def kernel(**inputs):
    in_maps = _prep_in_maps(**inputs)
    res = run_on_cores(in_maps)
    return assemble(res.results)



# revision 5
# speedup vs baseline: 1.0901x; 1.0901x over previous
"""ALIGN module kernel for 8 TRN2 NeuronCores (vocab-parallel, host-reduced).

Reference computation (B=4, S=576, Dv=1024, Dl=4096, V=32000):
    x  = vision_feats @ W1_w.T + W1_b          # [T=2304, Dl]
    xn = layernorm(x)                          # over Dl, no affine
    P  = softmax(xn @ W2_w.T, axis=-1)         # [T, V]
    F  = P @ llm_token_embed                   # [T, Dl]

Sharding: vocab dim of W2_w / llm_token_embed split across the 8 cores
(4000 rows each, zero-padded to 4096). Every core computes partial
numerators  N_c = exp(xn @ W2_c.T) @ E_c  and partial denominators
s_c = sum_v exp(...)  for ALL tokens; the host sums the 8 partials and
divides. NO ReduceScatter at all -- partials stream to DRAM as mm2
produces them, so the kernel tail is just the last store.

Token superblocks [256, 768, 768, 512]. Superblock 0 is computed
REPLICATED in stage A on every core (vision @ W1 for 256 tokens costs
~27us of PE and removes the first AllGather from the critical path);
superblocks 1-3 are token-parallel (each core computes its contiguous
1/8 chunk of xn) and AllGathered while phase B runs on earlier blocks.

W1/W1_b are COLUMN-CENTERED on the host so the LN mean is exactly 0:
stage A only needs E[x^2] (ones-vector matmul), Rsqrt, a rank-1
broadcast matmul and one elementwise multiply.

DMA queue separation (the previous version funneled 320MB through the
single qSync HWDGE queue -> head-of-line blocking at superblock
boundaries): W2 streams on qSync (nc.sync), emb + AllGather-result
loads on qScalar (nc.scalar), small stores (partial denominators) on
the gpsimd SWDGE, partial-numerator stores interleave on qScalar.
"""

import os
import sys

for _p in ("/opt/trn_rl_repo", "/root/.axon_site/_ro/trn_rl_repo"):
    if os.path.isdir(_p) and _p not in sys.path:
        sys.path.insert(0, _p)

import numpy as np
import ml_dtypes

from concourse import bass, bacc, mybir, tile
from concourse.bass_utils import run_bass_kernel_spmd

BF16NP = ml_dtypes.bfloat16
F32 = mybir.dt.float32
BF16 = mybir.dt.bfloat16

N_CORES = 8
T = 2304          # total tokens (B*S)
DV = 1024
DL = 4096
V_PAD = 4096      # padded vocab rows per core (4000 real + 96 zero pads)
NVT = V_PAD // 128  # 32 vocab tiles per core
NJ = DL // 128      # 32 contraction tiles
NK = DV // 128      # 8 stage-A contraction tiles
EC = 512          # matmul2 embedding-chunk width
N_EC = DL // EC   # 8 e-chunks

# superblocks: (start, size, per-core chunk size). sb0 replicated (tch=full).
SBS = [(0, 256, 256), (256, 768, 96), (1024, 768, 96), (1792, 512, 64)]
# mm1 moving-dim chunking per superblock (multiples of tch for the
# c-block-major xnt layout; sb0/sb3 fit one PSUM bank per chunk)
SB_CHUNKS = [[(0, 256)], [(0, 384), (384, 384)], [(0, 384), (384, 384)],
             [(0, 512)]]
# local vision column offset of each superblock's share
SH_COL = [0, 256, 352, 448]
XNT_ELEMS = 8 * NJ * 96   # flat per-partition extent of the xnt buffer

_NC_CACHE = None


def build():
    nc = bacc.Bacc("TRN2", target_bir_lowering=False, debug=False,
                   num_devices=N_CORES)
    rg = [list(range(N_CORES))]

    visionT = nc.dram_tensor("visionT", [DV, 512], BF16, kind="ExternalInput")
    w1t = nc.dram_tensor("w1t", [DV, DL], BF16, kind="ExternalInput")
    w1b = nc.dram_tensor("w1b", [128, NJ], F32, kind="ExternalInput")
    # [vt][p][j][vi]: per-partition unit-stride 8KB runs
    w2t = nc.dram_tensor("w2t", [NVT, 128, NJ, 128], BF16, kind="ExternalInput")
    # [e][p][vt][n]: per-partition unit-stride 16KB runs
    emb = nc.dram_tensor("emb", [N_EC, 128, NVT, EC], BF16,
                         kind="ExternalInput")
    ones_v = nc.dram_tensor("ones_v", [128, NVT, 1], BF16, kind="ExternalInput")
    # partial numerators / denominators; host sums over cores and divides
    out2 = nc.dram_tensor("out2", [T, DL], BF16, kind="ExternalOutput")
    out3 = nc.dram_tensor("out3", [T, 1], F32, kind="ExternalOutput")

    from contextlib import ExitStack
    with tile.TileContext(nc) as tc, ExitStack() as ctx:
        consts = ctx.enter_context(tc.tile_pool(name="consts", bufs=1))
        dram = ctx.enter_context(tc.tile_pool(name="dram", bufs=1, space="DRAM"))
        # phase-B-critical pools allocated BEFORE stage A so phase B's first
        # loads/matmuls are not gated on stage-A pool release
        w2_p = ctx.enter_context(tc.tile_pool(name="w2_p", bufs=3))
        xnt_p = ctx.enter_context(tc.tile_pool(name="xnt_p", bufs=1))
        l_ps = ctx.enter_context(tc.tile_pool(name="l_ps", bufs=3, space="PSUM"))

        onesv_sb = consts.tile([128, NVT, 1], BF16)
        nc.sync.dma_start(onesv_sb, ones_v[:])

        # per-superblock AllGather buffers (sb1-3); byte layout per core
        # block: [p][j][i] with dl = 128*j + p
        ag_ins = [None]
        ag_outs = [None]
        for si in (1, 2, 3):
            tch = SBS[si][2]
            gi = dram.tile([DL * tch], BF16, tag=f"ag_in_{si}",
                           name=f"ag_in_{si}")
            go = dram.tile([N_CORES * DL * tch], BF16, addr_space="Shared",
                           tag=f"ag_out_{si}", name=f"ag_out_{si}")
            ag_ins.append(gi)
            ag_outs.append(go)

        # flat xnt buffers (rotating): custom AP views per layout
        xnt_tiles = {}

        def xnt_alloc(si):
            t = xnt_p.tile([128, XNT_ELEMS], BF16, tag="xnt",
                           name=f"xnt_{si}")
            xnt_tiles[si] = t
            return t

        def xnt_rhs(si, j, c0, cw):
            t = xnt_tiles[si]
            tch = SBS[si][2]
            if si == 0:
                return bass.AP(tensor=t.tensor, offset=t.offset + j * 256 + c0,
                               ap=[list(t.ap[0]), [1, cw]])
            nc_blk = cw // tch
            return bass.AP(
                tensor=t.tensor,
                offset=t.offset + (c0 // tch) * (NJ * tch) + j * tch,
                ap=[list(t.ap[0]), [NJ * tch, nc_blk], [1, tch]])

        # preallocate + start the first w2 tiles right away (qSync, after
        # the small const; before the stage-A loads)
        w2_tiles = []

        def w2_alloc(si, vt):
            t = w2_p.tile([128, NJ, 128], BF16, tag="w2",
                          name=f"w2_{si}_{vt}")
            nc.sync.dma_start(t, w2t[vt][:])
            return t

        # ---------------- Stage A
        with ExitStack() as actx:
            sa = actx.enter_context(tc.tile_pool(name="stageA", bufs=1))
            sa2 = actx.enter_context(tc.tile_pool(name="stageA2", bufs=2))
            sq_p = actx.enter_context(tc.tile_pool(name="sq_p", bufs=3))
            psa = actx.enter_context(tc.tile_pool(name="psumA", bufs=1,
                                                  space="PSUM"))

            vt_sb = sa.tile([128, NK, 512], BF16)
            for k in range(NK):
                nc.sync.dma_start(vt_sb[:, k, :],
                                  visionT[128 * k:128 * (k + 1), :])
            w1t_sb = sa.tile([128, NK, DL], BF16)
            for mb in range(8):
                for k in range(NK):
                    nc.sync.dma_start(
                        w1t_sb[:, k, 512 * mb:512 * (mb + 1)],
                        w1t[128 * k:128 * (k + 1), 512 * mb:512 * (mb + 1)])
            b_cols = sa.tile([128, NJ], F32)
            nc.sync.dma_start(b_cols, w1b[:])
            onescol = sa.tile([128, 1], BF16)
            nc.vector.memset(onescol, 1.0)
            onesrow_f = sa.tile([1, 128], F32)
            nc.vector.memset(onesrow_f, 1.0)
            eps_sc = sa.tile([1, 1], F32)
            nc.vector.memset(eps_sc, 1e-5)

            # prefetch the first two w2 tiles behind the stage-A loads
            for vt in range(2):
                w2_tiles.append(w2_alloc(0, vt))

            def bc(t, rep, tch):
                # [128, tch] -> [128, rep, tch] stride-0 broadcast
                return bass.AP(tensor=t.tensor, offset=t.offset,
                               ap=[list(t.ap[0]), [0, rep], [1, tch]])

            sa_state = {}

            def sa_finish(si):
                # broadcast rstd, apply; ship + AllGather for shares, or
                # write xnt0 directly for the replicated sb0
                xt, rstd_row, tch = sa_state[si]
                rstdb_p = psa.tile([128, 256], F32, tag="rstdb",
                                   name=f"rstdb_{si}")
                nc.tensor.matmul(rstdb_p[:, :tch], lhsT=onesrow_f,
                                 rhs=rstd_row[:, :tch])
                rstdb = sa2.tile([128, 256], BF16, tag="rstdb_sb",
                                 name=f"rstdb_sb_{si}")
                nc.vector.tensor_copy(out=rstdb[:, :tch],
                                      in_=rstdb_p[:, :tch])
                if si == 0:
                    t = xnt_tiles[0]
                    for q in range(4):
                        dst = bass.AP(
                            tensor=t.tensor, offset=t.offset + q * 8 * 256,
                            ap=[list(t.ap[0]), [256, 8], [1, 256]])
                        nc.vector.tensor_mul(out=dst,
                                             in0=xt[:, 8 * q:8 * (q + 1), :],
                                             in1=bc(rstdb, 8, 256))
                    return
                xn_ch = sa2.tile([128, NJ, tch], BF16, tag=f"xn_{si}",
                                 name=f"xn_{si}", bufs=1)
                nc.vector.tensor_mul(out=xn_ch, in0=xt,
                                     in1=bc(rstdb, NJ, tch))
                for q in range(4):
                    nc.sync.dma_start(
                        bass.AP(tensor=ag_ins[si].tensor,
                                offset=ag_ins[si].offset + q * 8 * tch,
                                ap=[[NJ * tch, 128], [1, 8 * tch]]),
                        xn_ch[:, 8 * q:8 * (q + 1), :])
                nc.gpsimd.collective_compute(
                    "AllGather", mybir.AluOpType.bypass, replica_groups=rg,
                    ins=[ag_ins[si].opt()], outs=[ag_outs[si].opt()])

            order = [1, 2, 3, 0]
            for oi, si in enumerate(order):
                tch = SBS[si][2]
                c0 = SH_COL[si]
                if si == 0:
                    xnt_alloc(0)
                xt = sa2.tile([128, NJ, tch], BF16, tag=f"xt_{si}",
                              name=f"xt_{si}", bufs=1)
                s2p = psa.tile([1, 256], F32, tag="s2", name=f"s2_{si}",
                               bufs=2)
                sqs = {}

                def s2_step(m):
                    # E[x^2] accumulation, deferred 2 m-tiles so the PE
                    # never waits on the act->square chain
                    nc.tensor.matmul(s2p[:, :tch], lhsT=onescol,
                                     rhs=sqs.pop(m)[:, :tch],
                                     start=(m == 0), stop=(m == NJ - 1))

                for m in range(NJ):
                    xp = psa.tile([128, 256], F32, tag="xp",
                                  name=f"xp_{si}_{m}", bufs=2)
                    for k in range(NK):
                        nc.tensor.matmul(
                            xp[:, :tch],
                            lhsT=w1t_sb[:, k, 128 * m:128 * (m + 1)],
                            rhs=vt_sb[:, k, c0:c0 + tch],
                            start=(k == 0), stop=(k == NK - 1))
                    nc.scalar.activation(
                        out=xt[:, m, :], in_=xp[:, :tch],
                        func=mybir.ActivationFunctionType.Identity,
                        bias=b_cols[:, m:m + 1])
                    sq = sq_p.tile([128, 256], BF16, tag="sq",
                                   name=f"sq_{si}_{m}", bufs=4)
                    nc.vector.tensor_mul(out=sq[:, :tch], in0=xt[:, m, :],
                                         in1=xt[:, m, :])
                    sqs[m] = sq
                    if m >= 2:
                        s2_step(m - 2)
                    if oi > 0 and m == 16:
                        sa_finish(order[oi - 1])
                s2_step(NJ - 2)
                s2_step(NJ - 1)
                msq_row = sa2.tile([1, 256], F32, tag="msq",
                                   name=f"msq_{si}", bufs=1)
                nc.vector.tensor_scalar(
                    out=msq_row[:, :tch], in0=s2p[:, :tch],
                    scalar1=1.0 / DL, scalar2=None,
                    op0=mybir.AluOpType.mult)
                sd_row = sa2.tile([1, 256], F32, tag="sd",
                                  name=f"sd_{si}", bufs=1)
                nc.scalar.activation(
                    out=sd_row[:, :tch], in_=msq_row[:, :tch],
                    func=mybir.ActivationFunctionType.Sqrt, bias=eps_sc)
                rstd_row = sa2.tile([1, 256], F32, tag="rstd",
                                    name=f"rstd_{si}")
                nc.vector.reciprocal(out=rstd_row[:, :tch],
                                     in_=sd_row[:, :tch])
                sa_state[si] = (xt, rstd_row, tch)
            sa_finish(0)

        # ---------------- Phase B
        pt_p = ctx.enter_context(tc.tile_pool(name="pt_p", bufs=1))
        eb_p = ctx.enter_context(tc.tile_pool(name="eb_p", bufs=2))
        fs_p = ctx.enter_context(tc.tile_pool(name="fs_p", bufs=2))
        ss_p = ctx.enter_context(tc.tile_pool(name="ss_p", bufs=2))
        s_ps = ctx.enter_context(tc.tile_pool(name="s_ps", bufs=2, space="PSUM"))
        f_ps = ctx.enter_context(tc.tile_pool(name="f_ps", bufs=3, space="PSUM"))

        def make_xnt(si):
            # c-block-major loads from the AG output (128 x 6KB runs per
            # block) on the qScalar queue
            t = xnt_alloc(si)
            tch = SBS[si][2]
            for c in range(N_CORES):
                off = ag_outs[si].offset + c * 128 * NJ * tch
                dst = bass.AP(tensor=t.tensor,
                              offset=t.offset + c * NJ * tch,
                              ap=[list(t.ap[0]), [1, NJ * tch]])
                nc.scalar.dma_start(
                    dst, bass.AP(tensor=ag_outs[si].tensor, offset=off,
                                 ap=[[NJ * tch, 128], [1, NJ * tch]]))

        def eb_alloc(si, e):
            t = eb_p.tile([128, NVT, EC], BF16, tag="eb",
                          name=f"eb_{si}_{e}")
            nc.scalar.dma_start(t, emb[e][:])
            return t

        for si, (sb0, sbn, tch) in enumerate(SBS):
            n_tt = sbn // 128
            # prefetch this superblock's first two e-chunks (qScalar)
            ebs = {0: eb_alloc(si, 0), 1: eb_alloc(si, 1)}

            # matmul1: logitsT per v-tile, exp -> pt
            pt = pt_p.tile([128, NVT, 768], BF16, tag="pt", name=f"pt_{si}")
            for vt in range(NVT):
                if w2_tiles:
                    w2sb = w2_tiles.pop(0)
                else:
                    w2sb = w2_alloc(si, vt)
                for c0, cw in SB_CHUNKS[si]:
                    lp = l_ps.tile([128, 512], F32, tag="lp",
                                   name=f"lp_{si}_{vt}_{c0}")
                    for j in range(NJ):
                        nc.tensor.matmul(
                            lp[:, :cw], lhsT=w2sb[:, j, :],
                            rhs=xnt_rhs(si, j, c0, cw),
                            start=(j == 0), stop=(j == NJ - 1))
                    nc.scalar.activation(
                        out=pt[:, vt, c0:c0 + cw], in_=lp[:, :cw],
                        func=mybir.ActivationFunctionType.Exp)

            # queue next superblock's xnt loads (qScalar; AG long done)
            if si + 1 < len(SBS):
                make_xnt(si + 1)

            # partial softmax denominators for this superblock -> out3
            for tt in range(n_tt):
                sp = s_ps.tile([128, 1], F32, tag="sp", name=f"sp_{si}_{tt}")
                for vt in range(NVT):
                    nc.tensor.matmul(
                        sp, lhsT=pt[:, vt, 128 * tt:128 * (tt + 1)],
                        rhs=onesv_sb[:, vt, :],
                        start=(vt == 0), stop=(vt == NVT - 1))
                ss = ss_p.tile([128, 1], F32, tag="ss", name=f"ss_{si}_{tt}")
                nc.scalar.activation(
                    out=ss, in_=sp,
                    func=mybir.ActivationFunctionType.Identity)
                nc.gpsimd.dma_start(
                    out3[sb0 + 128 * tt:sb0 + 128 * (tt + 1), :], ss)

            # matmul2: partial F = pt.T @ emb per e-chunk -> out2 (bf16)
            for e in range(N_EC):
                eb = ebs.pop(e)
                for tt in range(n_tt):
                    fp = f_ps.tile([128, EC], F32, tag="fp",
                                   name=f"fp_{si}_{e}_{tt}")
                    for vt in range(NVT):
                        nc.tensor.matmul(
                            fp, lhsT=pt[:, vt, 128 * tt:128 * (tt + 1)],
                            rhs=eb[:, vt, :],
                            start=(vt == 0), stop=(vt == NVT - 1))
                    fs = fs_p.tile([128, EC], BF16, tag="fs",
                                   name=f"fs_{si}_{e}_{tt}")
                    nc.scalar.activation(
                        out=fs, in_=fp,
                        func=mybir.ActivationFunctionType.Identity)
                    nc.scalar.dma_start(
                        out2[sb0 + 128 * tt:sb0 + 128 * (tt + 1),
                             EC * e:EC * (e + 1)], fs)
                # issue the e+2 prefetch AFTER this iteration's fs stores:
                # its WAR wait (on this iteration's chains) must not
                # head-of-line-block the stores on the qScalar FIFO
                if e + 2 < N_EC:
                    ebs[e + 2] = eb_alloc(si, e + 2)

    nc.compile()
    return nc


def _get_nc():
    global _NC_CACHE
    if _NC_CACHE is None:
        _NC_CACHE = build()
    return _NC_CACHE


def _prep_in_maps(vision_feats, W1_w, W1_b, W2_w, llm_token_embed):
    vf = np.ascontiguousarray(np.asarray(vision_feats, np.float32)).reshape(
        T, DV)
    W1 = np.asarray(W1_w, np.float32)
    b1 = np.asarray(W1_b, np.float32).reshape(DL)
    # column-center W1/b over the Dl output dim: makes the LN mean exactly 0
    W1 = W1 - W1.mean(axis=0, keepdims=True)
    b1 = np.ascontiguousarray((b1 - b1.mean()).reshape(NJ, 128).T)
    W2 = np.asarray(W2_w, np.float32)
    E = np.asarray(llm_token_embed, np.float32)

    w1t = np.ascontiguousarray(W1.T).astype(BF16NP)
    v_loc = 32000 // N_CORES
    in_maps = []
    for c in range(N_CORES):
        # vision cols: [sb0 all 256 | own sb1 share | own sb2 | own sb3]
        tok = np.concatenate(
            [np.arange(0, 256)]
            + [np.arange(sb0 + tch * c, sb0 + tch * (c + 1))
               for sb0, _, tch in SBS[1:]])
        vT = np.ascontiguousarray(vf[tok].T).astype(BF16NP)
        w2p = np.zeros((V_PAD, DL), np.float32)
        w2p[:v_loc] = W2[v_loc * c:v_loc * (c + 1)]
        # [vt, p, j, vi] with p = d % 128, j = d // 128, vi = v % 128
        w2tt = w2p.T.reshape(NJ, 128, NVT, 128).transpose(2, 1, 0, 3).astype(
            BF16NP)
        ep = np.zeros((V_PAD, DL), np.float32)
        ep[:v_loc] = E[v_loc * c:v_loc * (c + 1)]
        # [e, p, vt, n] with p = v % 128, vt = v // 128, n = d % EC
        ebt = ep.reshape(NVT, 128, N_EC, EC).transpose(2, 1, 0, 3).astype(
            BF16NP)
        onesv = np.zeros((128, NVT, 1), np.float32)
        for vt in range(NVT):
            for p in range(128):
                if 128 * vt + p < v_loc:
                    onesv[p, vt, 0] = 1.0
        in_maps.append({
            "visionT": vT,
            "w1t": w1t,
            "w1b": b1,
            "w2t": np.ascontiguousarray(w2tt),
            "emb": np.ascontiguousarray(ebt),
            "ones_v": onesv.astype(BF16NP),
        })
    return in_maps


def run_on_cores(in_maps, trace=False, **kwargs):
    nc = _get_nc()
    return run_bass_kernel_spmd(nc, in_maps, core_ids=list(range(N_CORES)),
                                trace=trace, **kwargs)


def assemble(core_results):
    num = np.zeros((T, DL), np.float32)
    den = np.zeros((T, 1), np.float32)
    for c in range(N_CORES):
        num += np.asarray(core_results[c]["out2"]).astype(np.float32)
        den += np.asarray(core_results[c]["out3"])
    return (num / den).reshape(4, 576, DL)


def kernel(**inputs):
    in_maps = _prep_in_maps(**inputs)
    res = run_on_cores(in_maps)
    return assemble(res.results)


# revision 17
# speedup vs baseline: 1.1027x; 1.0116x over previous
"""ALIGN module kernel for 8 TRN2 NeuronCores (vocab-parallel, host-reduced).

Reference computation (B=4, S=576, Dv=1024, Dl=4096, V=32000):
    x  = vision_feats @ W1_w.T + W1_b          # [T=2304, Dl]
    xn = layernorm(x)                          # over Dl, no affine
    P  = softmax(xn @ W2_w.T, axis=-1)         # [T, V]
    F  = P @ llm_token_embed                   # [T, Dl]

Sharding: vocab dim of W2_w / llm_token_embed split across the 8 cores
(4000 rows each, zero-padded to 4096). Every core computes partial
numerators  N_c = exp(xn @ W2_c.T) @ E_c  and partial denominators
s_c = sum_v exp(...)  for ALL tokens; the host sums the 8 partials and
divides. NO ReduceScatter at all -- partials stream to DRAM as mm2
produces them, so the kernel tail is just the last store.

Token superblocks [256, 768, 768, 512]. Superblock 0 is computed
REPLICATED in stage A on every core (vision @ W1 for 256 tokens costs
~27us of PE and removes the first AllGather from the critical path);
superblocks 1-3 are token-parallel (each core computes its contiguous
1/8 chunk of xn) and AllGathered while phase B runs on earlier blocks.

W1/W1_b are COLUMN-CENTERED on the host so the LN mean is exactly 0:
stage A only needs E[x^2] (ones-vector matmul), Rsqrt, a rank-1
broadcast matmul and one elementwise multiply.

DMA queue separation (the previous version funneled 320MB through the
single qSync HWDGE queue -> head-of-line blocking at superblock
boundaries): W2 streams on qSync (nc.sync), emb + AllGather-result
loads on qScalar (nc.scalar), small stores (partial denominators) on
the gpsimd SWDGE, partial-numerator stores interleave on qScalar.
"""

import os
import sys

for _p in ("/opt/trn_rl_repo", "/root/.axon_site/_ro/trn_rl_repo"):
    if os.path.isdir(_p) and _p not in sys.path:
        sys.path.insert(0, _p)

import numpy as np
import ml_dtypes

from concourse import bass, bacc, mybir, tile
from concourse.bass_utils import run_bass_kernel_spmd

BF16NP = ml_dtypes.bfloat16
F32 = mybir.dt.float32
BF16 = mybir.dt.bfloat16

N_CORES = 8
T = 2304          # total tokens (B*S)
DV = 1024
DL = 4096
V_PAD = 4096      # padded vocab rows per core (4000 real + 96 zero pads)
NVT = V_PAD // 128  # 32 vocab tiles per core
NJ = DL // 128      # 32 contraction tiles
NK = DV // 128      # 8 stage-A contraction tiles
EC = 512          # matmul2 embedding-chunk width
N_EC = DL // EC   # 8 e-chunks

# superblocks: (start, size, per-core chunk size). sb0 replicated (tch=full).
SBS = [(0, 256, 256), (256, 768, 96), (1024, 768, 96), (1792, 512, 64)]
# mm1 moving-dim chunking per superblock (multiples of tch for the
# c-block-major xnt layout; sb0/sb3 fit one PSUM bank per chunk)
SB_CHUNKS = [[(0, 256)], [(0, 384), (384, 384)], [(0, 384), (384, 384)],
             [(0, 512)]]
# local vision column offset of each superblock's share
SH_COL = [0, 256, 352, 448]
XNT_ELEMS = 8 * NJ * 96   # flat per-partition extent of the xnt buffer

_NC_CACHE = None


def build():
    nc = bacc.Bacc("TRN2", target_bir_lowering=False, debug=False,
                   num_devices=N_CORES)
    rg = [list(range(N_CORES))]

    visionT = nc.dram_tensor("visionT", [DV, 512], BF16, kind="ExternalInput")
    w1t = nc.dram_tensor("w1t", [DV, DL], BF16, kind="ExternalInput")
    w1b = nc.dram_tensor("w1b", [128, NJ], F32, kind="ExternalInput")
    # [vt][p][j][vi]: per-partition unit-stride 8KB runs
    w2t = nc.dram_tensor("w2t", [NVT, 128, NJ, 128], BF16, kind="ExternalInput")
    # [e][p][vt][n]: per-partition unit-stride 16KB runs
    emb = nc.dram_tensor("emb", [N_EC, 128, NVT, EC], BF16,
                         kind="ExternalInput")
    ones_v = nc.dram_tensor("ones_v", [128, NVT, 1], BF16, kind="ExternalInput")
    # partial numerators / denominators; host sums over cores and divides
    out2 = nc.dram_tensor("out2", [T, DL], BF16, kind="ExternalOutput")
    out3 = nc.dram_tensor("out3", [T, 1], F32, kind="ExternalOutput")

    from contextlib import ExitStack
    with tile.TileContext(nc) as tc, ExitStack() as ctx:
        consts = ctx.enter_context(tc.tile_pool(name="consts", bufs=1))
        dram = ctx.enter_context(tc.tile_pool(name="dram", bufs=1, space="DRAM"))
        # phase-B-critical pools allocated BEFORE stage A so phase B's first
        # loads/matmuls are not gated on stage-A pool release
        w2_p = ctx.enter_context(tc.tile_pool(name="w2_p", bufs=3))
        xnt_p = ctx.enter_context(tc.tile_pool(name="xnt_p", bufs=1))
        l_ps = ctx.enter_context(tc.tile_pool(name="l_ps", bufs=3, space="PSUM"))

        onesv_sb = consts.tile([128, NVT, 1], BF16)
        nc.sync.dma_start(onesv_sb, ones_v[:])

        # per-superblock AllGather buffers (sb1-3); byte layout per core
        # block: [p][j][i] with dl = 128*j + p
        ag_ins = [None]
        ag_outs = [None]
        for si in (1, 2, 3):
            tch = SBS[si][2]
            gi = dram.tile([DL * tch], BF16, tag=f"ag_in_{si}",
                           name=f"ag_in_{si}")
            go = dram.tile([N_CORES * DL * tch], BF16, addr_space="Shared",
                           tag=f"ag_out_{si}", name=f"ag_out_{si}")
            ag_ins.append(gi)
            ag_outs.append(go)

        # flat xnt buffers (rotating): custom AP views per layout
        xnt_tiles = {}

        def xnt_alloc(si):
            t = xnt_p.tile([128, XNT_ELEMS], BF16, tag="xnt",
                           name=f"xnt_{si}")
            xnt_tiles[si] = t
            return t

        def xnt_rhs(si, j, c0, cw):
            t = xnt_tiles[si]
            tch = SBS[si][2]
            if si == 0:
                return bass.AP(tensor=t.tensor, offset=t.offset + j * 256 + c0,
                               ap=[list(t.ap[0]), [1, cw]])
            nc_blk = cw // tch
            return bass.AP(
                tensor=t.tensor,
                offset=t.offset + (c0 // tch) * (NJ * tch) + j * tch,
                ap=[list(t.ap[0]), [NJ * tch, nc_blk], [1, tch]])

        # preallocate + start the first w2 tiles right away (qSync, after
        # the small const; before the stage-A loads)
        w2_tiles = []

        def w2_alloc(si, vt, eng=None):
            t = w2_p.tile([128, NJ, 128], BF16, tag="w2",
                          name=f"w2_{si}_{vt}")
            (eng or nc.sync).dma_start(t, w2t[vt][:])
            return t

        # ---------------- Stage A
        with ExitStack() as actx:
            sa = actx.enter_context(tc.tile_pool(name="stageA", bufs=1))
            sa2 = actx.enter_context(tc.tile_pool(name="stageA2", bufs=2))
            sq_p = actx.enter_context(tc.tile_pool(name="sq_p", bufs=3))
            psa = actx.enter_context(tc.tile_pool(name="psumA", bufs=1,
                                                  space="PSUM"))

            # single-trigger loads: DMA-trigger instructions cost ~600ns
            # each on the issuing engine, so batch big and split across
            # both HWDGE queues (qSync + qScalar). vision + w1t own the
            # HBM first -- everything in stage A waits on them.
            vt_sb = sa.tile([128, NK, 512], BF16)
            nc.sync.dma_start(
                vt_sb,
                bass.AP(tensor=visionT.ap().tensor, offset=0,
                        ap=[[512, 128], [512 * 128, NK], [1, 512]]))
            w1t_sb = sa.tile([128, NK, DL], BF16)
            for k in range(NK):
                eng = nc.sync if k % 2 == 0 else nc.scalar
                eng.dma_start(w1t_sb[:, k, :],
                              w1t[128 * k:128 * (k + 1), :])
            b_cols = sa.tile([128, NJ], F32)
            nc.sync.dma_start(b_cols, w1b[:])
            onescol = sa.tile([128, 1], BF16)
            nc.vector.memset(onescol, 1.0)
            onesrow_f = sa.tile([1, 128], F32)
            nc.vector.memset(onesrow_f, 1.0)
            eps_sc = sa.tile([1, 1], F32)
            nc.vector.memset(eps_sc, 1e-5)

            # prefetch the first three w2 tiles behind the stage-A loads
            for vt in range(3):
                w2_tiles.append(w2_alloc(0, vt))

            def bc(t, rep, tch):
                # [128, tch] -> [128, rep, tch] stride-0 broadcast
                return bass.AP(tensor=t.tensor, offset=t.offset,
                               ap=[list(t.ap[0]), [0, rep], [1, tch]])

            # ONE joint m-loop over all 512 local token columns (sb0's 256
            # replicated tokens + the three shares): N=512 hides LDWEIGHTS
            # completely and there is a single act->square latency chain.
            # sb0's x goes straight into the xnt buffer (bf16); the rstd
            # scale is applied in-place at the end.
            xnt0 = xnt_alloc(0)

            def xnt0_row(m):
                return bass.AP(tensor=xnt0.tensor,
                               offset=xnt0.offset + m * 256,
                               ap=[list(xnt0.ap[0]), [1, 256]])

            xtsh = sa2.tile([128, NJ, 256], BF16, tag="xtsh", name="xtsh",
                            bufs=1)
            # (si, local col0, tch); sb0 is cols 0:256
            RANGES = [(0, 0, 256), (1, 256, 96), (2, 352, 96), (3, 448, 64)]
            # one joint E[x^2] chain over all 512 local token columns;
            # ranges slice the result afterward
            s2p = psa.tile([1, 512], F32, tag="s2", name="s2", bufs=1)
            sqs = {}

            def s2_step(m):
                nc.tensor.matmul(s2p, lhsT=onescol, rhs=sqs[m],
                                 start=(m == 0), stop=(m == NJ - 1))
                if m >= 2:
                    sqs.pop(m - 2)

            for m in range(NJ):
                xp = psa.tile([128, 512], F32, tag="xp", name=f"xp_{m}",
                              bufs=2)
                for k in range(NK):
                    nc.tensor.matmul(
                        xp, lhsT=w1t_sb[:, k, 128 * m:128 * (m + 1)],
                        rhs=vt_sb[:, k, :],
                        start=(k == 0), stop=(k == NK - 1))
                nc.scalar.activation(
                    out=xnt0_row(m), in_=xp[:, :256],
                    func=mybir.ActivationFunctionType.Identity,
                    bias=b_cols[:, m:m + 1])
                nc.scalar.activation(
                    out=xtsh[:, m, :], in_=xp[:, 256:],
                    func=mybir.ActivationFunctionType.Identity,
                    bias=b_cols[:, m:m + 1])
                sq = sq_p.tile([128, 512], BF16, tag="sq", name=f"sq_{m}",
                               bufs=4)
                nc.vector.tensor_mul(out=sq[:, :256], in0=xnt0_row(m),
                                     in1=xnt0_row(m))
                nc.vector.tensor_mul(out=sq[:, 256:], in0=xtsh[:, m, :],
                                     in1=xtsh[:, m, :])
                sqs[m] = sq
                if m >= 2:
                    s2_step(m - 2)
            s2_step(NJ - 2)
            s2_step(NJ - 1)

            # rstd chains for all 4 ranges (pipeline on DVE/ACT)
            rstds = []
            for r, (si, c0, tch) in enumerate(RANGES):
                msq_row = sa2.tile([1, 256], F32, tag="msq",
                                   name=f"msq_{r}", bufs=2)
                nc.vector.tensor_scalar(
                    out=msq_row[:, :tch], in0=s2p[:, c0:c0 + tch],
                    scalar1=1.0 / DL, scalar2=None,
                    op0=mybir.AluOpType.mult)
                sd_row = sa2.tile([1, 256], F32, tag="sd",
                                  name=f"sd_{r}", bufs=2)
                nc.scalar.activation(
                    out=sd_row[:, :tch], in_=msq_row[:, :tch],
                    func=mybir.ActivationFunctionType.Sqrt, bias=eps_sc)
                rstd_row = sa2.tile([1, 256], F32, tag="rstd",
                                    name=f"rstd_{r}", bufs=4)
                nc.vector.reciprocal(out=rstd_row[:, :tch],
                                     in_=sd_row[:, :tch])
                rstds.append(rstd_row)

            def sa_finish(r):
                # broadcast rstd, apply; sb0 multiplies xnt0 in place,
                # shares ship to their AllGather
                si, c0, tch = RANGES[r]
                rstd_row = rstds[r]
                # rides the xp ring (the m-loop is done by now)
                rstdb_p = psa.tile([128, 512], F32, tag="xp",
                                   name=f"rstdb_{r}", bufs=2)
                nc.tensor.matmul(rstdb_p[:, :tch], lhsT=onesrow_f,
                                 rhs=rstd_row[:, :tch])
                rstdb = sa2.tile([128, 256], BF16, tag="rstdb_sb",
                                 name=f"rstdb_sb_{r}", bufs=2)
                nc.vector.tensor_copy(out=rstdb[:, :tch],
                                      in_=rstdb_p[:, :tch])
                if si == 0:
                    for q in range(4):
                        dst = bass.AP(
                            tensor=xnt0.tensor,
                            offset=xnt0.offset + q * 8 * 256,
                            ap=[list(xnt0.ap[0]), [256, 8], [1, 256]])
                        nc.vector.tensor_mul(out=dst, in0=dst,
                                             in1=bc(rstdb, 8, 256))
                    return
                xn_ch = sa2.tile([128, NJ, tch], BF16, tag="xn",
                                 name=f"xn_{si}", bufs=1)
                nc.vector.tensor_mul(out=xn_ch,
                                     in0=xtsh[:, :, c0 - 256:c0 - 256 + tch],
                                     in1=bc(rstdb, NJ, tch))
                # gpsimd SWDGE: these stores are gated on the xn_ch multiply
                # (stage-A end) and would head-of-line-block the w2 stream
                # if queued on qSync
                for q in range(4):
                    nc.gpsimd.dma_start(
                        bass.AP(tensor=ag_ins[si].tensor,
                                offset=ag_ins[si].offset + q * 8 * tch,
                                ap=[[NJ * tch, 128], [1, 8 * tch]]),
                        xn_ch[:, 8 * q:8 * (q + 1), :])
                nc.gpsimd.collective_compute(
                    "AllGather", mybir.AluOpType.bypass, replica_groups=rg,
                    ins=[ag_ins[si].opt()], outs=[ag_outs[si].opt()])

            # sb0 first: mm1 can start the moment xnt0 is scaled; the
            # share AllGathers fire a few us later under mm1's cover
            for r in (0, 1, 2, 3):
                sa_finish(r)

        # ---------------- Phase B
        pt_p = ctx.enter_context(tc.tile_pool(name="pt_p", bufs=1))
        eb_p = ctx.enter_context(tc.tile_pool(name="eb_p", bufs=2))
        fs_p = ctx.enter_context(tc.tile_pool(name="fs_p", bufs=2))
        ss_p = ctx.enter_context(tc.tile_pool(name="ss_p", bufs=2))
        s_ps = ctx.enter_context(tc.tile_pool(name="s_ps", bufs=2, space="PSUM"))
        f_ps = ctx.enter_context(tc.tile_pool(name="f_ps", bufs=3, space="PSUM"))

        def make_xnt(si):
            # c-block-major loads from the AG output (128 x 6KB runs per
            # block) on the qScalar queue
            t = xnt_alloc(si)
            tch = SBS[si][2]
            for c in range(N_CORES):
                off = ag_outs[si].offset + c * 128 * NJ * tch
                dst = bass.AP(tensor=t.tensor,
                              offset=t.offset + c * NJ * tch,
                              ap=[list(t.ap[0]), [1, NJ * tch]])
                nc.scalar.dma_start(
                    dst, bass.AP(tensor=ag_outs[si].tensor, offset=off,
                                 ap=[[NJ * tch, 128], [1, NJ * tch]]))

        def eb_alloc(si, e):
            t = eb_p.tile([128, NVT, EC], BF16, tag="eb",
                          name=f"eb_{si}_{e}")
            nc.scalar.dma_start(t, emb[e][:])
            return t

        for si, (sb0, sbn, tch) in enumerate(SBS):
            n_tt = sbn // 128
            # prefetch this superblock's first two e-chunks (qScalar)
            ebs = {0: eb_alloc(si, 0), 1: eb_alloc(si, 1)}

            # matmul1: logitsT per v-tile, exp -> pt
            pt = pt_p.tile([128, NVT, 768], BF16, tag="pt", name=f"pt_{si}")
            for vt in range(NVT):
                if w2_tiles:
                    w2sb = w2_tiles.pop(0)
                else:
                    w2sb = w2_alloc(si, vt)
                for c0, cw in SB_CHUNKS[si]:
                    lp = l_ps.tile([128, 512], F32, tag="lp",
                                   name=f"lp_{si}_{vt}_{c0}")
                    for j in range(NJ):
                        nc.tensor.matmul(
                            lp[:, :cw], lhsT=w2sb[:, j, :],
                            rhs=xnt_rhs(si, j, c0, cw),
                            start=(j == 0), stop=(j == NJ - 1))
                    nc.scalar.activation(
                        out=pt[:, vt, c0:c0 + cw], in_=lp[:, :cw],
                        func=mybir.ActivationFunctionType.Exp)

            # queue next superblock's xnt loads (qScalar; AG long done)
            if si + 1 < len(SBS):
                make_xnt(si + 1)

            # partial softmax denominators for this superblock -> out3
            for tt in range(n_tt):
                sp = s_ps.tile([128, 1], F32, tag="sp", name=f"sp_{si}_{tt}")
                for vt in range(NVT):
                    nc.tensor.matmul(
                        sp, lhsT=pt[:, vt, 128 * tt:128 * (tt + 1)],
                        rhs=onesv_sb[:, vt, :],
                        start=(vt == 0), stop=(vt == NVT - 1))
                ss = ss_p.tile([128, 1], F32, tag="ss", name=f"ss_{si}_{tt}")
                nc.scalar.activation(
                    out=ss, in_=sp,
                    func=mybir.ActivationFunctionType.Identity)
                nc.gpsimd.dma_start(
                    out3[sb0 + 128 * tt:sb0 + 128 * (tt + 1), :], ss)

            # matmul2: partial F = pt.T @ emb per e-chunk -> out2 (bf16)
            for e in range(N_EC):
                eb = ebs.pop(e)
                for tt in range(n_tt):
                    fp = f_ps.tile([128, EC], F32, tag="fp",
                                   name=f"fp_{si}_{e}_{tt}")
                    for vt in range(NVT):
                        nc.tensor.matmul(
                            fp, lhsT=pt[:, vt, 128 * tt:128 * (tt + 1)],
                            rhs=eb[:, vt, :],
                            start=(vt == 0), stop=(vt == NVT - 1))
                    fs = fs_p.tile([128, EC], BF16, tag="fs",
                                   name=f"fs_{si}_{e}_{tt}")
                    nc.scalar.activation(
                        out=fs, in_=fp,
                        func=mybir.ActivationFunctionType.Identity)
                    # qSync is idle during mm2 (w2 streams only during mm1)
                    nc.sync.dma_start(
                        out2[sb0 + 128 * tt:sb0 + 128 * (tt + 1),
                             EC * e:EC * (e + 1)], fs)
                # issue the e+2 prefetch AFTER this iteration's fs stores:
                # its WAR wait (on this iteration's chains) must not
                # head-of-line-block the stores on the qScalar FIFO
                if e + 2 < N_EC:
                    ebs[e + 2] = eb_alloc(si, e + 2)

    nc.compile()
    return nc


def _get_nc():
    global _NC_CACHE
    if _NC_CACHE is None:
        _NC_CACHE = build()
    return _NC_CACHE


def _prep_in_maps(vision_feats, W1_w, W1_b, W2_w, llm_token_embed):
    vf = np.ascontiguousarray(np.asarray(vision_feats, np.float32)).reshape(
        T, DV)
    W1 = np.asarray(W1_w, np.float32)
    b1 = np.asarray(W1_b, np.float32).reshape(DL)
    # column-center W1/b over the Dl output dim: makes the LN mean exactly 0
    W1 = W1 - W1.mean(axis=0, keepdims=True)
    b1 = np.ascontiguousarray((b1 - b1.mean()).reshape(NJ, 128).T)
    W2 = np.asarray(W2_w, np.float32)
    E = np.asarray(llm_token_embed, np.float32)

    w1t = np.ascontiguousarray(W1.T).astype(BF16NP)
    v_loc = 32000 // N_CORES
    in_maps = []
    for c in range(N_CORES):
        # vision cols: [sb0 all 256 | own sb1 share | own sb2 | own sb3]
        tok = np.concatenate(
            [np.arange(0, 256)]
            + [np.arange(sb0 + tch * c, sb0 + tch * (c + 1))
               for sb0, _, tch in SBS[1:]])
        vT = np.ascontiguousarray(vf[tok].T).astype(BF16NP)
        w2p = np.zeros((V_PAD, DL), np.float32)
        w2p[:v_loc] = W2[v_loc * c:v_loc * (c + 1)]
        # [vt, p, j, vi] with p = d % 128, j = d // 128, vi = v % 128
        w2tt = w2p.T.reshape(NJ, 128, NVT, 128).transpose(2, 1, 0, 3).astype(
            BF16NP)
        ep = np.zeros((V_PAD, DL), np.float32)
        ep[:v_loc] = E[v_loc * c:v_loc * (c + 1)]
        # [e, p, vt, n] with p = v % 128, vt = v // 128, n = d % EC
        ebt = ep.reshape(NVT, 128, N_EC, EC).transpose(2, 1, 0, 3).astype(
            BF16NP)
        onesv = np.zeros((128, NVT, 1), np.float32)
        for vt in range(NVT):
            for p in range(128):
                if 128 * vt + p < v_loc:
                    onesv[p, vt, 0] = 1.0
        in_maps.append({
            "visionT": vT,
            "w1t": w1t,
            "w1b": b1,
            "w2t": np.ascontiguousarray(w2tt),
            "emb": np.ascontiguousarray(ebt),
            "ones_v": onesv.astype(BF16NP),
        })
    return in_maps


def run_on_cores(in_maps, trace=False, **kwargs):
    nc = _get_nc()
    return run_bass_kernel_spmd(nc, in_maps, core_ids=list(range(N_CORES)),
                                trace=trace, **kwargs)


def assemble(core_results):
    num = np.zeros((T, DL), np.float32)
    den = np.zeros((T, 1), np.float32)
    for c in range(N_CORES):
        num += np.asarray(core_results[c]["out2"]).astype(np.float32)
        den += np.asarray(core_results[c]["out3"])
    return (num / den).reshape(4, 576, DL)


def kernel(**inputs):
    in_maps = _prep_in_maps(**inputs)
    res = run_on_cores(in_maps)
    return assemble(res.results)
